# revision 15
# baseline (speedup 1.0000x reference)
"""Trainium2 Bass kernel for DimeNet-style Interaction block (gnn_message_passing).

Strategy (8 NeuronCores, SPMD, no collectives). The end-to-end metric is
dominated by the host<->device tunnel (~79 MB/s H2D, ~50 MB/s D2H), so the
design minimizes shipped bytes:
  - Host: sort triplets by edge_index_to; split edges into 8 equal contiguous
    slices (one per core). Each core gets its triplet run, grouped into blocks
    of <=384 triplets (3 subtiles of 128) covering <=128 consecutive edges.
    Host pre-gathers per-triplet inputs: x rows as int8 (one global scale,
    folded into w_from on the host), radial rows and sbf = spherical@w_sbf in
    bf16. The device program is fully dense - no indirect DMA.
  - Device per core (bf16 matmuls, fp32 PSUM):
      x_kj^T = silu(w_from'^T @ xg^T + b) * (w_rbf^T @ radial^T)
      per 128-triplet subtile:
        tmp   = x_kj_tile^T.T @ W2             [128,1024] PSUM
        tmp'j = tmp_j * sbf[:,j]               (ACT/DVE scale, bf16)
        S     = (iota == to_local)             (DVE is_equal, bf16)
        agg  += S^T @ tmp'_j                   (8 bf16 MMs, PSUM-accumulated)
      drain agg -> PE transpose -> slot-layout agg^T [128, NB*128] bf16
      epilogue on slot columns: h = silu(x@w_to+b)+agg; residual stack (bf16).
  - Output shipped bf16 [128, W_S] per core; host compacts slots -> edge rows.
"""
import os
import numpy as np
import ml_dtypes

BF16 = ml_dtypes.bfloat16

H, B, NR, NS = 128, 8, 6, 7
P = 128
NSUB = 3
BLK_T = NSUB * P     # triplets per block
SLOT_W = 128         # block edge-coverage <= SLOT_W
N_CORES = 8
EP_N = 512           # epilogue column-block width

_PROG_CACHE = {}


def _enable_jax_compile_cache():
    try:
        import jax
        jax.config.update("jax_compilation_cache_dir", "/tmp/jax_cache")
        jax.config.update("jax_persistent_cache_min_compile_time_secs", 0)
        jax.config.update("jax_persistent_cache_min_entry_size_bytes", 0)
    except Exception:
        pass


_enable_jax_compile_cache()


def make_blocks(ct, local_end):
    """Greedy blocks over sorted local to-indices ct: each block takes whole
    runs of equal ct while (value - cov_lo) < SLOT_W and count <= BLK_T."""
    n = len(ct)
    blocks = []
    cov_lo = 0
    if n:
        run_starts = np.flatnonzero(np.r_[True, ct[1:] != ct[:-1]])
        run_vals = ct[run_starts]
        run_ends = np.r_[run_starts[1:], n]
        nruns = len(run_vals)
        r = 0
        while r < nruns:
            v0 = int(run_vals[r])
            if v0 - cov_lo >= SLOT_W:
                ts = int(run_starts[r])
                blocks.append((ts, ts, cov_lo))
                cov_lo += SLOT_W
                continue
            start_t = int(run_starts[r])
            r_val = int(np.searchsorted(run_vals, cov_lo + SLOT_W, side="left"))
            r_cnt = int(np.searchsorted(run_ends, start_t + BLK_T, side="right"))
            r_next = max(min(r_val, r_cnt), r + 1)
            te = int(run_ends[r_next - 1])
            assert te - start_t <= BLK_T, "edge in-degree exceeds BLK_T"
            blocks.append((start_t, te, cov_lo))
            cov_lo = int(run_vals[r_next - 1]) + 1
            r = r_next
    while cov_lo < local_end:
        blocks.append((n, n, cov_lo))
        cov_lo = min(cov_lo + SLOT_W, local_end)
    return blocks


def host_prep(x, radial, sph, e_from, e_to, w_sbf, x_scale):
    E_ = x.shape[0]
    perm = np.argsort(e_to, kind='stable')
    to_s = e_to[perm].astype(np.int64)
    from_s = e_from[perm].astype(np.int64)

    epc = (E_ + N_CORES - 1) // N_CORES
    bounds = np.searchsorted(to_s, [c * epc for c in range(N_CORES + 1)])

    # global source arrays (converted once)
    xq = np.clip(np.rint(x * (127.0 / x_scale)), -127, 127).astype(np.int8)
    rad16 = radial.astype(BF16)
    sbf_all = (sph @ w_sbf).astype(BF16)          # [T, B]
    x16 = x.astype(BF16)

    cores = []
    for c in range(N_CORES):
        t0, t1 = bounds[c], bounds[c + 1]
        e_lo = c * epc
        e_hi = min((c + 1) * epc, E_)
        ct = to_s[t0:t1] - e_lo
        blocks = make_blocks(ct, e_hi - e_lo)
        cores.append(dict(e_lo=e_lo, e_hi=e_hi, ct=ct, cf=from_s[t0:t1],
                          psl=perm[t0:t1], blocks=blocks))

    NB = max(max(len(c['blocks']) for c in cores), 2)
    if NB % 2:
        NB += 1
    T_pad = NB * BLK_T
    W_S = NB * SLOT_W

    for core in cores:
        blocks = core['blocks']
        ct, cf, psl = core['ct'], core['cf'], core['psl']
        e_lo, e_hi = core['e_lo'], core['e_hi']
        local_end = e_hi - e_lo
        n = len(ct)
        while len(blocks) < NB:
            blocks.append((n, n, local_end))
        barr = np.asarray(blocks, np.int64).reshape(NB, 3)
        ts_a, te_a, cov_lo_arr = barr[:, 0], barr[:, 1], barr[:, 2]
        cnt_a = te_a - ts_a
        # nonempty blocks tile [0, n) contiguously -> src order is identity
        dst = np.repeat(BLK_T * np.arange(NB) - ts_a, cnt_a) + np.arange(n)

        xg8 = np.zeros((T_pad, H), np.int8)
        radg = np.zeros((T_pad, NR), BF16)
        sbfg = np.zeros((T_pad, B), BF16)
        tol = np.zeros((T_pad,), np.float32)
        xg8[dst] = xq[cf]
        radg[dst] = rad16[cf]
        sbfg[dst] = sbf_all[psl]
        tol[dst] = (ct - np.repeat(cov_lo_arr, cnt_a)).astype(np.float32)

        nxt = np.r_[cov_lo_arr[1:], local_end]
        cov_w_arr = np.maximum(0, np.minimum(nxt, local_end) - cov_lo_arr)

        x_slots = np.zeros((W_S, H), BF16)
        for b in range(NB):
            lo, w = int(cov_lo_arr[b]), int(cov_w_arr[b])
            if w > 0:
                x_slots[b * SLOT_W: b * SLOT_W + w] = x16[e_lo + lo: e_lo + lo + w]

        core['xg_T'] = np.ascontiguousarray(xg8.T)
        core['radg_T'] = np.ascontiguousarray(radg.T)
        # per-subtile sbf columns: [128, NSUB*NB*B]
        core['sbf_cols'] = np.ascontiguousarray(
            sbfg.reshape(NSUB * NB, P, B).transpose(1, 0, 2).reshape(P, NSUB * NB * B))
        core['tol_cols'] = np.ascontiguousarray(tol.reshape(NSUB * NB, P).T)
        core['x_slots_T'] = np.ascontiguousarray(x_slots.T)
        core['cov_lo'] = cov_lo_arr
        core['cov_w'] = cov_w_arr
    return cores, dict(NB=NB, T_pad=T_pad, W_S=W_S, epc=epc)


def build_program(NB, T_pad, W_S):
    import concourse.bass as bass
    import concourse.tile as tile
    from concourse import bacc, mybir

    KPART = os.environ.get("KPART", "all")

    f32 = mybir.dt.float32
    bf16 = mybir.dt.bfloat16
    i8 = mybir.dt.int8
    AF = mybir.ActivationFunctionType
    ALU = mybir.AluOpType

    f32r = mybir.dt.float32r

    nc = bacc.Bacc(None, target_bir_lowering=False)
    xg_d = nc.dram_tensor("xg_T", [P, T_pad], i8, kind="ExternalInput")
    radg_d = nc.dram_tensor("radg_T", [NR, T_pad], bf16, kind="ExternalInput")
    sbf_d = nc.dram_tensor("sbf_cols", [P, NSUB * NB * B], bf16,
                           kind="ExternalInput")
    x_slots_d = nc.dram_tensor("x_slots_T", [P, W_S], bf16, kind="ExternalInput")
    MW = 2 * P + 9 + NSUB * NB
    cmisc_d = nc.dram_tensor("cmisc", [P, MW], f32, kind="ExternalInput")
    CW = H + B * H + H
    cw_d = nc.dram_tensor("cw", [P, CW], bf16, kind="ExternalInput")
    cwf_d = nc.dram_tensor("cwf", [P, 8 * H], f32, kind="ExternalInput")
    out_d = nc.dram_tensor("out_T", [P, W_S], bf16, kind="ExternalOutput")

    with tile.TileContext(nc) as tc:
        with (
            tc.tile_pool(name="consts", bufs=1) as cp,
            tc.tile_pool(name="persist", bufs=1) as pp,
        ):
            cmisc_t = cp.tile([P, MW], f32)
            nc.gpsimd.dma_start(out=cmisc_t[:], in_=cmisc_d[:, :])
            cw_t = cp.tile([P, CW], bf16)
            nc.gpsimd.dma_start(out=cw_t[:], in_=cw_d[:, :])
            sbf16_t = cp.tile([P, NSUB * NB * B], bf16)
            nc.gpsimd.dma_start(out=sbf16_t[:], in_=sbf_d[:, :])
            x_sb = cp.tile([P, W_S], bf16)
            nc.gpsimd.dma_start(out=x_sb[:], in_=x_slots_d[:, :])
            cwf_t = cp.tile([P, 8 * H], f32r)
            nc.gpsimd.dma_start(out=cwf_t[:], in_=cwf_d[:, :].bitcast(f32r))
            sbf_f = cp.tile([P, NSUB * NB * B], f32)
            nc.vector.tensor_copy(out=sbf_f[:], in_=sbf16_t[:])
            aggT_big = pp.tile([P, W_S], f32)

            iota_t = cmisc_t[:, 0:P]
            ident_t = cmisc_t[:, P:2 * P]
            bias_t = cmisc_t[:, 2 * P:2 * P + 9]
            tol_t = cmisc_t[:, 2 * P + 9:MW]
            w_from_t = cw_t[:, 0:H]
            W2_t = cw_t[:, H:H + B * H]
            epw_t = cwf_t
            w_rbf_t = cw_t[0:NR, H + B * H:CW]
            b_from = bias_t[:, 0:1]

            # ---------------- main loop ----------------
            with (
                tc.tile_pool(name="mio", bufs=4) as mio,
                tc.tile_pool(name="mwork", bufs=3) as mwork,
                tc.tile_pool(name="ptmp", bufs=1, space="PSUM") as ptmp,
                tc.tile_pool(name="pxk", bufs=1, space="PSUM") as pxk,
                tc.tile_pool(name="pagg", bufs=2, space="PSUM") as pagg,
                tc.tile_pool(name="psmall", bufs=1, space="PSUM") as psmall,
            ):
                for b in range(NB if KPART in ("all", "main") else 0):
                    c0 = b * BLK_T
                    xg8 = mio.tile([P, BLK_T], i8, tag="xg8")
                    nc.gpsimd.dma_start(out=xg8[:], in_=xg_d[:, c0:c0 + BLK_T])
                    rad = mio.tile([NR, BLK_T], bf16, tag="rad")
                    nc.gpsimd.dma_start(out=rad[:], in_=radg_d[:, c0:c0 + BLK_T])
                    xgc = mwork.tile([P, BLK_T], bf16, tag="xgc")
                    nc.vector.tensor_copy(out=xgc[:], in_=xg8[:])

                    xkj_p = pxk.tile([P, BLK_T], f32, tag="xkj_p")
                    nc.tensor.matmul(out=xkj_p[:], lhsT=w_from_t, rhs=xgc[:],
                                     start=True, stop=True)
                    rbf_p = pxk.tile([P, BLK_T], f32, tag="rbf_p")
                    nc.tensor.matmul(out=rbf_p[:], lhsT=w_rbf_t, rhs=rad[:],
                                     start=True, stop=True)
                    xkj_s = mwork.tile([P, BLK_T], f32, tag="xkj_s")
                    nc.scalar.activation(out=xkj_s[:], in_=xkj_p[:], func=AF.Silu,
                                         bias=b_from, scale=1.0)
                    xkj = mwork.tile([P, BLK_T], bf16, tag="xkj")
                    nc.vector.tensor_tensor(out=xkj[:], in0=xkj_s[:], in1=rbf_p[:],
                                            op=ALU.mult)

                    agg_p = pagg.tile([P, P], f32, tag="agg")
                    for s in range(NSUB):
                        w0 = s * P
                        sc0 = (NSUB * b + s) * B
                        tmpA = ptmp.tile([P, 4 * H], f32, tag="tmpA")
                        nc.tensor.matmul(out=tmpA[:], lhsT=xkj[:, w0:w0 + P],
                                         rhs=W2_t[:, 0:4 * H], start=True, stop=True)
                        tmpB = ptmp.tile([P, 4 * H], f32, tag="tmpB")
                        nc.tensor.matmul(out=tmpB[:], lhsT=xkj[:, w0:w0 + P],
                                         rhs=W2_t[:, 4 * H:8 * H], start=True,
                                         stop=True)

                        S = mwork.tile([P, P], bf16, tag="S")
                        nc.vector.tensor_tensor(
                            out=S[:],
                            in0=tol_t[:, NSUB * b + s: NSUB * b + s + 1]
                                .to_broadcast([P, P]),
                            in1=iota_t, op=ALU.is_equal)
                        tmpS = mwork.tile([P, B * H], bf16, tag="tmpS")
                        for j in range(B):
                            src = tmpA[:, j * H:(j + 1) * H] if j < 4 else \
                                  tmpB[:, (j - 4) * H:(j - 3) * H]
                            dst = tmpS[:, j * H:(j + 1) * H]
                            sc = sbf_f[:, sc0 + j:sc0 + j + 1]
                            if j % 2 == 0:
                                nc.scalar.activation(out=dst, in_=src, func=AF.Copy,
                                                     scale=sc)
                            else:
                                nc.vector.tensor_tensor(
                                    out=dst, in0=src,
                                    in1=sc.to_broadcast([P, H]), op=ALU.mult)
                        for j in range(B):
                            nc.tensor.matmul(out=agg_p[:], lhsT=S[:],
                                             rhs=tmpS[:, j * H:(j + 1) * H],
                                             start=(s == 0 and j == 0),
                                             stop=(s == NSUB - 1 and j == B - 1),
                                             skip_group_check=True)
                    agg_s = mwork.tile([P, P], f32, tag="agg_s")
                    nc.scalar.activation(out=agg_s[:], in_=agg_p[:], func=AF.Copy)
                    aggT_p = psmall.tile([P, P], f32, tag="aggT_p")
                    nc.tensor.transpose(out=aggT_p[:], in_=agg_s[:],
                                        identity=ident_t)
                    nc.vector.tensor_copy(
                        out=aggT_big[:, b * SLOT_W:(b + 1) * SLOT_W],
                        in_=aggT_p[:])

            # -------- epilogue (baseline f32r structure, bf16 I/O) --------
            with (
                tc.tile_pool(name="ework", bufs=2) as ework,
                tc.tile_pool(name="epsum", bufs=4, space="PSUM") as epsum,
            ):
                def ep_mm(lhs_idx, rhs_ap, n):
                    pt = epsum.tile([P, EP_N], f32, tag="ep_p")
                    nc.tensor.matmul(out=pt[:, :n],
                                     lhsT=epw_t[:, lhs_idx * H:(lhs_idx + 1) * H],
                                     rhs=rhs_ap, start=True, stop=True)
                    return pt

                def ep_silu(pt, bias_idx, n, tag, dt_=f32):
                    t = ework.tile([P, EP_N], dt_, tag=tag)
                    nc.scalar.activation(out=t[:, :n], in_=pt[:, :n], func=AF.Silu,
                                         bias=bias_t[:, bias_idx:bias_idx + 1],
                                         scale=1.0)
                    return t

                n_ep = (W_S + EP_N - 1) // EP_N
                if KPART == "main":
                    for eb in range(n_ep):
                        c0 = eb * EP_N
                        n = min(EP_N, W_S - c0)
                        nc.gpsimd.dma_start(out=out_d[:, c0:c0 + n],
                                            in_=aggT_big[:, c0:c0 + n])
                    n_ep = 0
                for eb in range(n_ep):
                    c0 = eb * EP_N
                    n = min(EP_N, W_S - c0)
                    x_t = ework.tile([P, EP_N], f32r, tag="x_t")
                    nc.vector.tensor_copy(out=x_t[:, :n], in_=x_sb[:, c0:c0 + n])
                    # h = silu(x@w_to+b_to) + agg
                    pt = ep_mm(0, x_t[:, :n], n)
                    xji = ep_silu(pt, 1, n, "xji")
                    h = ework.tile([P, EP_N], f32r, tag="h")
                    nc.vector.tensor_tensor(out=h[:, :n], in0=xji[:, :n],
                                            in1=aggT_big[:, c0:c0 + n], op=ALU.add)
                    # rb residual
                    t1 = ep_silu(ep_mm(1, h[:, :n], n), 2, n, "t1", f32r)
                    t2 = ep_silu(ep_mm(2, t1[:, :n], n), 3, n, "t2")
                    h2 = ework.tile([P, EP_N], f32r, tag="h2")
                    nc.vector.tensor_tensor(out=h2[:, :n], in0=h[:, :n],
                                            in1=t2[:, :n], op=ALU.add)
                    # lin + skip x
                    l1 = ep_silu(ep_mm(3, h2[:, :n], n), 4, n, "l1")
                    h3 = ework.tile([P, EP_N], f32r, tag="h3")
                    nc.vector.tensor_tensor(out=h3[:, :n], in0=l1[:, :n],
                                            in1=x_t[:, :n], op=ALU.add)
                    # ra residuals x2
                    t3 = ep_silu(ep_mm(4, h3[:, :n], n), 5, n, "t3", f32r)
                    t4 = ep_silu(ep_mm(5, t3[:, :n], n), 6, n, "t4")
                    h4 = ework.tile([P, EP_N], f32r, tag="h4")
                    nc.vector.tensor_tensor(out=h4[:, :n], in0=h3[:, :n],
                                            in1=t4[:, :n], op=ALU.add)
                    t5 = ep_silu(ep_mm(6, h4[:, :n], n), 7, n, "t5", f32r)
                    t6 = ep_silu(ep_mm(7, t5[:, :n], n), 8, n, "t6")
                    h5 = ework.tile([P, EP_N], bf16, tag="h5")
                    nc.vector.tensor_tensor(out=h5[:, :n], in0=h4[:, :n],
                                            in1=t6[:, :n], op=ALU.add)
                    nc.gpsimd.dma_start(out=out_d[:, c0:c0 + n], in_=h5[:, :n])
    nc.compile()
    return nc


def kernel(x, radial_basis, spherical_basis, edge_index_from, edge_index_to,
           w_rbf, w_sbf, w_from, b_from, w_to, b_to, W,
           rb_w, rb_b, lin_w, lin_b, ra_w, ra_b):
    from concourse.bass_utils import run_bass_kernel_spmd

    x = np.asarray(x, np.float32)
    radial = np.asarray(radial_basis, np.float32)
    sph = np.asarray(spherical_basis, np.float32)
    e_from = np.asarray(edge_index_from)
    e_to = np.asarray(edge_index_to)
    in_dtype = np.asarray(x).dtype

    x_scale = float(np.abs(x).max()) or 1.0
    cores, meta = host_prep(x, radial, sph, e_from, e_to,
                            np.asarray(w_sbf, np.float32), x_scale)
    NB, T_pad, W_S = meta['NB'], meta['T_pad'], meta['W_S']

    W_np = np.asarray(W, np.float32)
    W2 = np.ascontiguousarray(W_np.transpose(2, 1, 0).reshape(H, B * H))
    ep_w = np.concatenate([
        np.asarray(w_to, np.float32),
        np.asarray(rb_w, np.float32)[0, 0], np.asarray(rb_w, np.float32)[0, 1],
        np.asarray(lin_w, np.float32),
        np.asarray(ra_w, np.float32)[0, 0], np.asarray(ra_w, np.float32)[0, 1],
        np.asarray(ra_w, np.float32)[1, 0], np.asarray(ra_w, np.float32)[1, 1],
    ], axis=1)
    biases = np.stack([
        np.asarray(b_from, np.float32), np.asarray(b_to, np.float32),
        np.asarray(rb_b, np.float32)[0, 0], np.asarray(rb_b, np.float32)[0, 1],
        np.asarray(lin_b, np.float32),
        np.asarray(ra_b, np.float32)[0, 0], np.asarray(ra_b, np.float32)[0, 1],
        np.asarray(ra_b, np.float32)[1, 0], np.asarray(ra_b, np.float32)[1, 1],
    ], axis=1).astype(np.float32)
    iota = np.tile(np.arange(P, dtype=np.float32), (P, 1))
    ident = np.eye(P, dtype=np.float32)

    CW = H + B * H + H
    cw = np.zeros((P, CW), np.float32)
    cw[:, 0:H] = np.asarray(w_from, np.float32) * (x_scale / 127.0)
    cw[:, H:H + B * H] = W2
    cw[0:NR, H + B * H:CW] = np.asarray(w_rbf, np.float32)
    cw16 = np.ascontiguousarray(cw.astype(BF16))
    cwf = np.ascontiguousarray(ep_w.astype(np.float32))

    key = (NB, T_pad, W_S)
    nc = _PROG_CACHE.get(key)
    if nc is None:
        nc = build_program(NB, T_pad, W_S)
        _PROG_CACHE[key] = nc

    MW = 2 * P + 9 + NSUB * NB
    in_maps = []
    for core in cores:
        cmisc = np.zeros((P, MW), np.float32)
        cmisc[:, 0:P] = iota
        cmisc[:, P:2 * P] = ident
        cmisc[:, 2 * P:2 * P + 9] = biases
        cmisc[:, 2 * P + 9:MW] = core['tol_cols']
        in_maps.append({
            "xg_T": core['xg_T'], "radg_T": core['radg_T'],
            "sbf_cols": core['sbf_cols'], "x_slots_T": core['x_slots_T'],
            "cmisc": np.ascontiguousarray(cmisc), "cw": cw16, "cwf": cwf,
        })
    res = run_bass_kernel_spmd(nc, in_maps, core_ids=list(range(N_CORES)))
    kernel._last_results = res
    if os.environ.get("KERNEL_EXEC_TWICE"):
        import time as _time
        os.environ["BASS_NEVER_TRACE"] = "1"
        try:
            t0 = _time.perf_counter()
            run_bass_kernel_spmd(nc, in_maps, core_ids=list(range(N_CORES)))
            kernel._exec2_s = _time.perf_counter() - t0
        finally:
            os.environ.pop("BASS_NEVER_TRACE", None)

    E_ = x.shape[0]
    out = np.zeros((E_, H), np.float32)
    for core, om in zip(cores, res.results):
        hT = np.asarray(om["out_T"], dtype=np.float32)
        for b in range(NB):
            lo, w = int(core['cov_lo'][b]), int(core['cov_w'][b])
            if w > 0:
                out[core['e_lo'] + lo: core['e_lo'] + lo + w] = \
                    hT[:, b * SLOT_W: b * SLOT_W + w].T
    return out.astype(in_dtype, copy=False)


# revision 25
# speedup vs baseline: 1.0529x; 1.0529x over previous
"""Trainium2 Bass kernel for DimeNet-style Interaction block (gnn_message_passing).

Strategy (8 NeuronCores, SPMD, no collectives). The end-to-end metric is
dominated by the host<->device tunnel (~79 MB/s H2D, ~50 MB/s D2H), so the
design minimizes shipped bytes:
  - Host: sort triplets by edge_index_to; split edges into 8 equal contiguous
    slices (one per core). Each core gets its triplet run, grouped into blocks
    of <=384 triplets (3 subtiles of 128) covering <=128 consecutive edges.
    Host pre-gathers per-triplet inputs: x rows as int8 (one global scale,
    folded into w_from on the host), radial rows and sbf = spherical@w_sbf in
    bf16. The device program is fully dense - no indirect DMA.
  - Device per core (bf16 matmuls, fp32 PSUM):
      x_kj^T = silu(w_from'^T @ xg^T + b) * (w_rbf^T @ radial^T)
      per 128-triplet subtile:
        tmp   = x_kj_tile^T.T @ W2             [128,1024] PSUM
        tmp'j = tmp_j * sbf[:,j]               (ACT/DVE scale, bf16)
        S     = (iota == to_local)             (DVE is_equal, bf16)
        agg  += S^T @ tmp'_j                   (8 bf16 MMs, PSUM-accumulated)
      drain agg -> PE transpose -> slot-layout agg^T [128, NB*128] bf16
      epilogue on slot columns: h = silu(x@w_to+b)+agg; residual stack (bf16).
  - Output shipped bf16 [128, W_S] per core; host compacts slots -> edge rows.
"""
import os
import numpy as np
import ml_dtypes

BF16 = ml_dtypes.bfloat16

H, B, NR, NS = 128, 8, 6, 7
P = 128
NSUB = 3
BLK_T = NSUB * P     # triplets per block
SLOT_W = 128         # block edge-coverage <= SLOT_W
N_CORES = 8
EP_N = 512           # epilogue column-block width

_PROG_CACHE = {}


def _enable_jax_compile_cache():
    try:
        import jax
        jax.config.update("jax_compilation_cache_dir", "/tmp/jax_cache")
        jax.config.update("jax_persistent_cache_min_compile_time_secs", 0)
        jax.config.update("jax_persistent_cache_min_entry_size_bytes", 0)
    except Exception:
        pass


_enable_jax_compile_cache()


def make_blocks(ct, local_end):
    """Greedy blocks over sorted local to-indices ct: each block takes whole
    runs of equal ct while (value - cov_lo) < SLOT_W and count <= BLK_T."""
    n = len(ct)
    blocks = []
    cov_lo = 0
    if n:
        run_starts = np.flatnonzero(np.r_[True, ct[1:] != ct[:-1]])
        run_vals = ct[run_starts]
        run_ends = np.r_[run_starts[1:], n]
        nruns = len(run_vals)
        r = 0
        while r < nruns:
            v0 = int(run_vals[r])
            if v0 - cov_lo >= SLOT_W:
                ts = int(run_starts[r])
                blocks.append((ts, ts, cov_lo))
                cov_lo += SLOT_W
                continue
            start_t = int(run_starts[r])
            r_val = int(np.searchsorted(run_vals, cov_lo + SLOT_W, side="left"))
            r_cnt = int(np.searchsorted(run_ends, start_t + BLK_T, side="right"))
            r_next = max(min(r_val, r_cnt), r + 1)
            te = int(run_ends[r_next - 1])
            assert te - start_t <= BLK_T, "edge in-degree exceeds BLK_T"
            blocks.append((start_t, te, cov_lo))
            cov_lo = int(run_vals[r_next - 1]) + 1
            r = r_next
    while cov_lo < local_end:
        blocks.append((n, n, cov_lo))
        cov_lo = min(cov_lo + SLOT_W, local_end)
    return blocks


def host_prep(x, radial, sph, e_from, e_to, w_sbf, x_scale, rad_scale,
              sbf_scale_out):
    E_ = x.shape[0]
    perm = np.argsort(e_to, kind='stable')
    to_s = e_to[perm].astype(np.int64)
    from_s = e_from[perm].astype(np.int64)

    epc = (E_ + N_CORES - 1) // N_CORES
    bounds = np.searchsorted(to_s, [c * epc for c in range(N_CORES + 1)])

    # global source arrays (converted once)
    xq = np.clip(np.rint(x * (127.0 / x_scale)), -127, 127).astype(np.int8)
    rad16 = np.clip(np.rint(radial * (127.0 / rad_scale)), -127, 127).astype(np.int8)
    sbf_f32 = sph @ w_sbf                          # [T, B]
    s_sbf = float(np.abs(sbf_f32).max()) or 1.0
    sbf_scale_out.append(s_sbf)
    sbf_all = np.clip(np.rint(sbf_f32 * (127.0 / s_sbf)), -127, 127).astype(np.int8)
    x16 = x.astype(BF16)

    cores = []
    for c in range(N_CORES):
        t0, t1 = bounds[c], bounds[c + 1]
        e_lo = c * epc
        e_hi = min((c + 1) * epc, E_)
        ct = to_s[t0:t1] - e_lo
        blocks = make_blocks(ct, e_hi - e_lo)
        cores.append(dict(e_lo=e_lo, e_hi=e_hi, ct=ct, cf=from_s[t0:t1],
                          psl=perm[t0:t1], blocks=blocks))

    NB = max(max(len(c['blocks']) for c in cores), 2)
    if NB % 2:
        NB += 1
    T_pad = NB * BLK_T
    W_S = NB * SLOT_W

    for core in cores:
        blocks = core['blocks']
        ct, cf, psl = core['ct'], core['cf'], core['psl']
        e_lo, e_hi = core['e_lo'], core['e_hi']
        local_end = e_hi - e_lo
        n = len(ct)
        while len(blocks) < NB:
            blocks.append((n, n, local_end))
        barr = np.asarray(blocks, np.int64).reshape(NB, 3)
        ts_a, te_a, cov_lo_arr = barr[:, 0], barr[:, 1], barr[:, 2]
        cnt_a = te_a - ts_a
        # nonempty blocks tile [0, n) contiguously -> src order is identity
        dst = np.repeat(BLK_T * np.arange(NB) - ts_a, cnt_a) + np.arange(n)

        xg8 = np.zeros((T_pad, H), np.int8)
        radg = np.zeros((T_pad, NR), np.int8)
        sbfg = np.zeros((T_pad, B), np.int8)
        tol = np.zeros((T_pad,), np.float32)
        xg8[dst] = xq[cf]
        radg[dst] = rad16[cf]
        sbfg[dst] = sbf_all[psl]
        tol[dst] = (ct - np.repeat(cov_lo_arr, cnt_a)).astype(np.float32)

        nxt = np.r_[cov_lo_arr[1:], local_end]
        cov_w_arr = np.maximum(0, np.minimum(nxt, local_end) - cov_lo_arr)

        x_slots = np.zeros((W_S, H), BF16)
        for b in range(NB):
            lo, w = int(cov_lo_arr[b]), int(cov_w_arr[b])
            if w > 0:
                x_slots[b * SLOT_W: b * SLOT_W + w] = x16[e_lo + lo: e_lo + lo + w]

        # gi8 = [xg_T | per-subtile sbf columns], one int8 tensor
        sbf_cols = sbfg.reshape(NSUB * NB, P, B).transpose(1, 0, 2) \
                       .reshape(P, NSUB * NB * B)
        core['gi8'] = np.ascontiguousarray(
            np.concatenate([xg8.T, sbf_cols], axis=1))
        core['radg_T'] = np.ascontiguousarray(radg.T)
        core['tol_cols'] = np.ascontiguousarray(tol.reshape(NSUB * NB, P).T)
        core['x_slots_T'] = np.ascontiguousarray(x_slots.T)
        core['cov_lo'] = cov_lo_arr
        core['cov_w'] = cov_w_arr
    return cores, dict(NB=NB, T_pad=T_pad, W_S=W_S, epc=epc)


def build_program(NB, T_pad, W_S):
    import concourse.bass as bass
    import concourse.tile as tile
    from concourse import bacc, mybir

    KPART = os.environ.get("KPART", "all")

    f32 = mybir.dt.float32
    bf16 = mybir.dt.bfloat16
    i8 = mybir.dt.int8
    AF = mybir.ActivationFunctionType
    ALU = mybir.AluOpType

    f32r = mybir.dt.float32r

    SBW = NSUB * NB * B
    nc = bacc.Bacc(None, target_bir_lowering=False)
    gi8_d = nc.dram_tensor("gi8", [P, T_pad + SBW], i8, kind="ExternalInput")
    radg_d = nc.dram_tensor("radg_T", [NR, T_pad], i8, kind="ExternalInput")
    MW = 2 * P + 9 + NSUB * NB
    cmisc_d = nc.dram_tensor("cmisc", [P, MW], f32, kind="ExternalInput")
    CW = H + B * H + H + 8 * H
    cwx_d = nc.dram_tensor("cwx", [P, CW + W_S], bf16, kind="ExternalInput")
    out_d = nc.dram_tensor("out_T", [P, W_S], bf16, kind="ExternalOutput")

    with tile.TileContext(nc) as tc:
        with (
            tc.tile_pool(name="consts", bufs=1) as cp,
            tc.tile_pool(name="persist", bufs=1) as pp,
        ):
            cmisc_t = cp.tile([P, MW], f32)
            nc.gpsimd.dma_start(out=cmisc_t[:], in_=cmisc_d[:, :])
            cwx_t = cp.tile([P, CW + W_S], bf16)
            nc.gpsimd.dma_start(out=cwx_t[:], in_=cwx_d[:, :])
            sbf8_t = cp.tile([P, SBW], i8)
            nc.gpsimd.dma_start(out=sbf8_t[:], in_=gi8_d[:, T_pad:T_pad + SBW])
            sbf_f = cp.tile([P, SBW], f32)
            nc.vector.tensor_copy(out=sbf_f[:], in_=sbf8_t[:])
            cwf_t = cp.tile([P, 8 * H], f32r)
            nc.vector.tensor_copy(out=cwf_t[:],
                                  in_=cwx_t[:, H + B * H + H:CW])
            aggT_big = pp.tile([P, W_S], f32)

            iota_t = cmisc_t[:, 0:P]
            ident_t = cmisc_t[:, P:2 * P]
            bias_t = cmisc_t[:, 2 * P:2 * P + 9]
            tol_t = cmisc_t[:, 2 * P + 9:MW]
            w_from_t = cwx_t[:, 0:H]
            W2_t = cwx_t[:, H:H + B * H]
            w_rbf_t = cwx_t[0:NR, H + B * H:H + B * H + H]
            epw_t = cwf_t
            x_sb = cwx_t[:, CW:CW + W_S]
            b_from = bias_t[:, 0:1]

            # ---------------- main loop ----------------
            with (
                tc.tile_pool(name="mio", bufs=4) as mio,
                tc.tile_pool(name="mwork", bufs=3) as mwork,
                tc.tile_pool(name="ptmp", bufs=1, space="PSUM") as ptmp,
                tc.tile_pool(name="pxk", bufs=1, space="PSUM") as pxk,
                tc.tile_pool(name="pagg", bufs=2, space="PSUM") as pagg,
                tc.tile_pool(name="psmall", bufs=1, space="PSUM") as psmall,
            ):
                for b in range(NB if KPART in ("all", "main") else 0):
                    c0 = b * BLK_T
                    xg8 = mio.tile([P, BLK_T], i8, tag="xg8")
                    nc.gpsimd.dma_start(out=xg8[:], in_=gi8_d[:, c0:c0 + BLK_T])
                    rad8 = mio.tile([NR, BLK_T], i8, tag="rad8")
                    nc.gpsimd.dma_start(out=rad8[:], in_=radg_d[:, c0:c0 + BLK_T])
                    rad = mwork.tile([NR, BLK_T], bf16, tag="rad")
                    nc.vector.tensor_copy(out=rad[:], in_=rad8[:])
                    xgc = mwork.tile([P, BLK_T], bf16, tag="xgc")
                    nc.vector.tensor_copy(out=xgc[:], in_=xg8[:])

                    xkj_p = pxk.tile([P, BLK_T], f32, tag="xkj_p")
                    nc.tensor.matmul(out=xkj_p[:], lhsT=w_from_t, rhs=xgc[:],
                                     start=True, stop=True)
                    rbf_p = pxk.tile([P, BLK_T], f32, tag="rbf_p")
                    nc.tensor.matmul(out=rbf_p[:], lhsT=w_rbf_t, rhs=rad[:],
                                     start=True, stop=True)
                    xkj_s = mwork.tile([P, BLK_T], f32, tag="xkj_s")
                    nc.scalar.activation(out=xkj_s[:], in_=xkj_p[:], func=AF.Silu,
                                         bias=b_from, scale=1.0)
                    xkj = mwork.tile([P, BLK_T], bf16, tag="xkj")
                    nc.vector.tensor_tensor(out=xkj[:], in0=xkj_s[:], in1=rbf_p[:],
                                            op=ALU.mult)

                    agg_p = pagg.tile([P, P], f32, tag="agg")
                    for s in range(NSUB):
                        w0 = s * P
                        sc0 = (NSUB * b + s) * B
                        tmpA = ptmp.tile([P, 4 * H], f32, tag="tmpA")
                        nc.tensor.matmul(out=tmpA[:], lhsT=xkj[:, w0:w0 + P],
                                         rhs=W2_t[:, 0:4 * H], start=True, stop=True)
                        tmpB = ptmp.tile([P, 4 * H], f32, tag="tmpB")
                        nc.tensor.matmul(out=tmpB[:], lhsT=xkj[:, w0:w0 + P],
                                         rhs=W2_t[:, 4 * H:8 * H], start=True,
                                         stop=True)

                        S = mwork.tile([P, P], bf16, tag="S")
                        nc.vector.tensor_tensor(
                            out=S[:],
                            in0=tol_t[:, NSUB * b + s: NSUB * b + s + 1]
                                .to_broadcast([P, P]),
                            in1=iota_t, op=ALU.is_equal)
                        tmpS = mwork.tile([P, B * H], bf16, tag="tmpS")
                        for half, tsrc in ((0, tmpA), (1, tmpB)):
                            dst3 = tmpS[:, half * 4 * H:(half + 1) * 4 * H] \
                                .rearrange("p (b h) -> p b h", b=4)
                            src3 = tsrc[:].rearrange("p (b h) -> p b h", b=4)
                            sc3 = sbf_f[:, sc0 + half * 4:sc0 + half * 4 + 4] \
                                .unsqueeze(2).to_broadcast([P, 4, H])
                            nc.vector.tensor_tensor(out=dst3, in0=src3, in1=sc3,
                                                    op=ALU.mult)
                        for j in range(B):
                            nc.tensor.matmul(out=agg_p[:], lhsT=S[:],
                                             rhs=tmpS[:, j * H:(j + 1) * H],
                                             start=(s == 0 and j == 0),
                                             stop=(s == NSUB - 1 and j == B - 1),
                                             skip_group_check=True)
                    agg_s = mwork.tile([P, P], f32, tag="agg_s")
                    nc.scalar.activation(out=agg_s[:], in_=agg_p[:], func=AF.Copy)
                    aggT_p = psmall.tile([P, P], f32, tag="aggT_p")
                    nc.tensor.transpose(out=aggT_p[:], in_=agg_s[:],
                                        identity=ident_t)
                    nc.vector.tensor_copy(
                        out=aggT_big[:, b * SLOT_W:(b + 1) * SLOT_W],
                        in_=aggT_p[:])

            # -------- epilogue (baseline f32r structure, bf16 I/O) --------
            with (
                tc.tile_pool(name="ework", bufs=2) as ework,
                tc.tile_pool(name="epsum", bufs=4, space="PSUM") as epsum,
            ):
                def ep_mm(lhs_idx, rhs_ap, n):
                    pt = epsum.tile([P, EP_N], f32, tag="ep_p")
                    nc.tensor.matmul(out=pt[:, :n],
                                     lhsT=epw_t[:, lhs_idx * H:(lhs_idx + 1) * H],
                                     rhs=rhs_ap, start=True, stop=True)
                    return pt

                def ep_silu(pt, bias_idx, n, tag, dt_=f32):
                    t = ework.tile([P, EP_N], dt_, tag=tag)
                    nc.scalar.activation(out=t[:, :n], in_=pt[:, :n], func=AF.Silu,
                                         bias=bias_t[:, bias_idx:bias_idx + 1],
                                         scale=1.0)
                    return t

                n_ep = (W_S + EP_N - 1) // EP_N
                if KPART == "main":
                    for eb in range(n_ep):
                        c0 = eb * EP_N
                        n = min(EP_N, W_S - c0)
                        nc.gpsimd.dma_start(out=out_d[:, c0:c0 + n],
                                            in_=aggT_big[:, c0:c0 + n])
                    n_ep = 0
                for eb in range(n_ep):
                    c0 = eb * EP_N
                    n = min(EP_N, W_S - c0)
                    x_t = ework.tile([P, EP_N], f32r, tag="x_t")
                    nc.vector.tensor_copy(out=x_t[:, :n], in_=x_sb[:, c0:c0 + n])
                    # h = silu(x@w_to+b_to) + agg
                    pt = ep_mm(0, x_t[:, :n], n)
                    xji = ep_silu(pt, 1, n, "xji")
                    h = ework.tile([P, EP_N], f32r, tag="h")
                    nc.vector.tensor_tensor(out=h[:, :n], in0=xji[:, :n],
                                            in1=aggT_big[:, c0:c0 + n], op=ALU.add)
                    # rb residual
                    t1 = ep_silu(ep_mm(1, h[:, :n], n), 2, n, "t1", f32r)
                    t2 = ep_silu(ep_mm(2, t1[:, :n], n), 3, n, "t2")
                    h2 = ework.tile([P, EP_N], f32r, tag="h2")
                    nc.vector.tensor_tensor(out=h2[:, :n], in0=h[:, :n],
                                            in1=t2[:, :n], op=ALU.add)
                    # lin + skip x
                    l1 = ep_silu(ep_mm(3, h2[:, :n], n), 4, n, "l1")
                    h3 = ework.tile([P, EP_N], f32r, tag="h3")
                    nc.vector.tensor_tensor(out=h3[:, :n], in0=l1[:, :n],
                                            in1=x_t[:, :n], op=ALU.add)
                    # ra residuals x2
                    t3 = ep_silu(ep_mm(4, h3[:, :n], n), 5, n, "t3", f32r)
                    t4 = ep_silu(ep_mm(5, t3[:, :n], n), 6, n, "t4")
                    h4 = ework.tile([P, EP_N], f32r, tag="h4")
                    nc.vector.tensor_tensor(out=h4[:, :n], in0=h3[:, :n],
                                            in1=t4[:, :n], op=ALU.add)
                    t5 = ep_silu(ep_mm(6, h4[:, :n], n), 7, n, "t5", f32r)
                    t6 = ep_silu(ep_mm(7, t5[:, :n], n), 8, n, "t6")
                    h5 = ework.tile([P, EP_N], bf16, tag="h5")
                    nc.vector.tensor_tensor(out=h5[:, :n], in0=h4[:, :n],
                                            in1=t6[:, :n], op=ALU.add)
                    nc.gpsimd.dma_start(out=out_d[:, c0:c0 + n], in_=h5[:, :n])
    nc.compile()
    return nc


def kernel(x, radial_basis, spherical_basis, edge_index_from, edge_index_to,
           w_rbf, w_sbf, w_from, b_from, w_to, b_to, W,
           rb_w, rb_b, lin_w, lin_b, ra_w, ra_b):
    from concourse.bass_utils import run_bass_kernel_spmd

    x = np.asarray(x, np.float32)
    radial = np.asarray(radial_basis, np.float32)
    sph = np.asarray(spherical_basis, np.float32)
    e_from = np.asarray(edge_index_from)
    e_to = np.asarray(edge_index_to)
    in_dtype = np.asarray(x).dtype

    x_scale = float(np.abs(x).max()) or 1.0
    rad_scale = float(np.abs(radial).max()) or 1.0
    sbf_scale_out = []
    cores, meta = host_prep(x, radial, sph, e_from, e_to,
                            np.asarray(w_sbf, np.float32), x_scale, rad_scale,
                            sbf_scale_out)
    sbf_scale = sbf_scale_out[0]
    NB, T_pad, W_S = meta['NB'], meta['T_pad'], meta['W_S']

    W_np = np.asarray(W, np.float32)
    W2 = np.ascontiguousarray(W_np.transpose(2, 1, 0).reshape(H, B * H))
    ep_w = np.concatenate([
        np.asarray(w_to, np.float32),
        np.asarray(rb_w, np.float32)[0, 0], np.asarray(rb_w, np.float32)[0, 1],
        np.asarray(lin_w, np.float32),
        np.asarray(ra_w, np.float32)[0, 0], np.asarray(ra_w, np.float32)[0, 1],
        np.asarray(ra_w, np.float32)[1, 0], np.asarray(ra_w, np.float32)[1, 1],
    ], axis=1)
    biases = np.stack([
        np.asarray(b_from, np.float32), np.asarray(b_to, np.float32),
        np.asarray(rb_b, np.float32)[0, 0], np.asarray(rb_b, np.float32)[0, 1],
        np.asarray(lin_b, np.float32),
        np.asarray(ra_b, np.float32)[0, 0], np.asarray(ra_b, np.float32)[0, 1],
        np.asarray(ra_b, np.float32)[1, 0], np.asarray(ra_b, np.float32)[1, 1],
    ], axis=1).astype(np.float32)
    iota = np.tile(np.arange(P, dtype=np.float32), (P, 1))
    ident = np.eye(P, dtype=np.float32)

    CW = H + B * H + H + 8 * H
    cw = np.zeros((P, CW), np.float32)
    cw[:, 0:H] = np.asarray(w_from, np.float32) * (x_scale / 127.0)
    cw[:, H:H + B * H] = W2 * (sbf_scale / 127.0)
    cw[0:NR, H + B * H:H + B * H + H] = \
        np.asarray(w_rbf, np.float32) * (rad_scale / 127.0)
    cw[:, H + B * H + H:CW] = ep_w
    cw16 = cw.astype(BF16)

    key = (NB, T_pad, W_S)
    nc = _PROG_CACHE.get(key)
    if nc is None:
        nc = build_program(NB, T_pad, W_S)
        _PROG_CACHE[key] = nc

    MW = 2 * P + 9 + NSUB * NB
    in_maps = []
    for core in cores:
        cmisc = np.zeros((P, MW), np.float32)
        cmisc[:, 0:P] = iota
        cmisc[:, P:2 * P] = ident
        cmisc[:, 2 * P:2 * P + 9] = biases
        cmisc[:, 2 * P + 9:MW] = core['tol_cols']
        in_maps.append({
            "gi8": core['gi8'], "radg_T": core['radg_T'],
            "cmisc": np.ascontiguousarray(cmisc),
            "cwx": np.ascontiguousarray(
                np.concatenate([cw16, core['x_slots_T']], axis=1)),
        })
    res = run_bass_kernel_spmd(nc, in_maps, core_ids=list(range(N_CORES)))
    kernel._last_results = res
    if os.environ.get("KERNEL_EXEC_TWICE"):
        import time as _time
        os.environ["BASS_NEVER_TRACE"] = "1"
        try:
            t0 = _time.perf_counter()
            run_bass_kernel_spmd(nc, in_maps, core_ids=list(range(N_CORES)))
            kernel._exec2_s = _time.perf_counter() - t0
        finally:
            os.environ.pop("BASS_NEVER_TRACE", None)

    E_ = x.shape[0]
    out = np.zeros((E_, H), np.float32)
    for core, om in zip(cores, res.results):
        hT = np.asarray(om["out_T"], dtype=np.float32)
        for b in range(NB):
            lo, w = int(core['cov_lo'][b]), int(core['cov_w'][b])
            if w > 0:
                out[core['e_lo'] + lo: core['e_lo'] + lo + w] = \
                    hT[:, b * SLOT_W: b * SLOT_W + w].T
    return out.astype(in_dtype, copy=False)


# revision 32
# speedup vs baseline: 1.5683x; 1.4894x over previous
"""Trainium2 Bass kernel for DimeNet-style Interaction block (gnn_message_passing).

Strategy (8 NeuronCores, SPMD, no collectives). The end-to-end metric is
dominated by the host<->device tunnel (~79 MB/s H2D, ~50 MB/s D2H), so the
design minimizes shipped bytes:
  - Host: sort triplets by edge_index_to; split edges into 8 equal contiguous
    slices (one per core). Each core gets its triplet run, grouped into blocks
    of <=384 triplets (3 subtiles of 128) covering <=128 consecutive edges.
    Host pre-gathers per-triplet inputs: x rows as int8 (one global scale,
    folded into w_from on the host), radial rows and sbf = spherical@w_sbf in
    bf16. The device program is fully dense - no indirect DMA.
  - Device per core (bf16 matmuls, fp32 PSUM):
      x_kj^T = silu(w_from'^T @ xg^T + b) * (w_rbf^T @ radial^T)
      per 128-triplet subtile:
        tmp   = x_kj_tile^T.T @ W2             [128,1024] PSUM
        tmp'j = tmp_j * sbf[:,j]               (ACT/DVE scale, bf16)
        S     = (iota == to_local)             (DVE is_equal, bf16)
        agg  += S^T @ tmp'_j                   (8 bf16 MMs, PSUM-accumulated)
      drain agg -> PE transpose -> slot-layout agg^T [128, NB*128] bf16
      epilogue on slot columns: h = silu(x@w_to+b)+agg; residual stack (bf16).
  - Output shipped bf16 [128, W_S] per core; host compacts slots -> edge rows.
"""
import os
import numpy as np
import ml_dtypes

BF16 = ml_dtypes.bfloat16

H, B, NR, NS = 128, 8, 6, 7
P = 128
NSUB = 3
BLK_T = NSUB * P     # triplets per block
SLOT_W = 128         # block edge-coverage <= SLOT_W
N_CORES = 8
EP_N = 512           # epilogue column-block width

_PROG_CACHE = {}


def _enable_jax_compile_cache():
    try:
        import jax
        jax.config.update("jax_compilation_cache_dir", "/tmp/jax_cache")
        jax.config.update("jax_persistent_cache_min_compile_time_secs", 0)
        jax.config.update("jax_persistent_cache_min_entry_size_bytes", 0)
    except Exception:
        pass


_enable_jax_compile_cache()


def make_blocks(ct, local_end):
    """Greedy blocks over sorted local to-indices ct: each block takes whole
    runs of equal ct while (value - cov_lo) < SLOT_W and count <= BLK_T."""
    n = len(ct)
    blocks = []
    cov_lo = 0
    if n:
        run_starts = np.flatnonzero(np.r_[True, ct[1:] != ct[:-1]])
        run_vals = ct[run_starts]
        run_ends = np.r_[run_starts[1:], n]
        nruns = len(run_vals)
        r = 0
        while r < nruns:
            v0 = int(run_vals[r])
            if v0 - cov_lo >= SLOT_W:
                ts = int(run_starts[r])
                blocks.append((ts, ts, cov_lo))
                cov_lo += SLOT_W
                continue
            start_t = int(run_starts[r])
            r_val = int(np.searchsorted(run_vals, cov_lo + SLOT_W, side="left"))
            r_cnt = int(np.searchsorted(run_ends, start_t + BLK_T, side="right"))
            r_next = max(min(r_val, r_cnt), r + 1)
            te = int(run_ends[r_next - 1])
            assert te - start_t <= BLK_T, "edge in-degree exceeds BLK_T"
            blocks.append((start_t, te, cov_lo))
            cov_lo = int(run_vals[r_next - 1]) + 1
            r = r_next
    while cov_lo < local_end:
        blocks.append((n, n, cov_lo))
        cov_lo = min(cov_lo + SLOT_W, local_end)
    return blocks


def host_prep(x, radial, sph, e_from, e_to, w_sbf, x_scale, rad_scale,
              sbf_scale_out):
    E_ = x.shape[0]
    perm = np.argsort(e_to, kind='stable')
    to_s = e_to[perm].astype(np.int64)
    from_s = e_from[perm].astype(np.int64)

    epc = (E_ + N_CORES - 1) // N_CORES
    bounds = np.searchsorted(to_s, [c * epc for c in range(N_CORES + 1)])

    # global source arrays (converted once)
    xq = np.clip(np.rint(x * (127.0 / x_scale)), -127, 127).astype(np.int8)
    rad16 = np.clip(np.rint(radial * (127.0 / rad_scale)), -127, 127).astype(np.int8)
    sbf_f32 = sph @ w_sbf                          # [T, B]
    s_sbf = float(np.abs(sbf_f32).max()) or 1.0
    sbf_scale_out.append(s_sbf)
    sbf_all = np.clip(np.rint(sbf_f32 * (127.0 / s_sbf)), -127, 127).astype(np.int8)

    cores = []
    for c in range(N_CORES):
        t0, t1 = bounds[c], bounds[c + 1]
        e_lo = c * epc
        e_hi = min((c + 1) * epc, E_)
        ct = to_s[t0:t1] - e_lo
        blocks = make_blocks(ct, e_hi - e_lo)
        cores.append(dict(e_lo=e_lo, e_hi=e_hi, ct=ct, cf=from_s[t0:t1],
                          psl=perm[t0:t1], blocks=blocks))

    NB = max(max(len(c['blocks']) for c in cores), 2)
    if NB % 2:
        NB += 1
    T_pad = NB * BLK_T
    W_S = NB * SLOT_W

    for core in cores:
        blocks = core['blocks']
        ct, cf, psl = core['ct'], core['cf'], core['psl']
        e_lo, e_hi = core['e_lo'], core['e_hi']
        local_end = e_hi - e_lo
        n = len(ct)
        while len(blocks) < NB:
            blocks.append((n, n, local_end))
        barr = np.asarray(blocks, np.int64).reshape(NB, 3)
        ts_a, te_a, cov_lo_arr = barr[:, 0], barr[:, 1], barr[:, 2]
        cnt_a = te_a - ts_a
        # nonempty blocks tile [0, n) contiguously -> src order is identity
        dst = np.repeat(BLK_T * np.arange(NB) - ts_a, cnt_a) + np.arange(n)

        xg8 = np.zeros((T_pad, H), np.int8)
        radg = np.zeros((T_pad, NR), np.int8)
        sbfg = np.zeros((T_pad, B), np.int8)
        tol = np.zeros((T_pad,), np.float32)
        xg8[dst] = xq[cf]
        radg[dst] = rad16[cf]
        sbfg[dst] = sbf_all[psl]
        tol[dst] = (ct - np.repeat(cov_lo_arr, cnt_a)).astype(np.float32)

        nxt = np.r_[cov_lo_arr[1:], local_end]
        cov_w_arr = np.maximum(0, np.minimum(nxt, local_end) - cov_lo_arr)

        x_slots = np.zeros((W_S, H), np.int8)
        for b in range(NB):
            lo, w = int(cov_lo_arr[b]), int(cov_w_arr[b])
            if w > 0:
                x_slots[b * SLOT_W: b * SLOT_W + w] = xq[e_lo + lo: e_lo + lo + w]

        # gi8 = [xg_T | per-subtile sbf columns | x slots], one int8 tensor
        sbf_cols = sbfg.reshape(NSUB * NB, P, B).transpose(1, 0, 2) \
                       .reshape(P, NSUB * NB * B)
        core['gi8'] = np.ascontiguousarray(
            np.concatenate([xg8.T, sbf_cols, x_slots.T], axis=1))
        core['radg_T'] = np.ascontiguousarray(radg.T)
        core['tol_cols'] = np.ascontiguousarray(tol.reshape(NSUB * NB, P).T)
        core['cov_lo'] = cov_lo_arr
        core['cov_w'] = cov_w_arr
    return cores, dict(NB=NB, T_pad=T_pad, W_S=W_S, epc=epc)


def build_program(NB, T_pad, W_S):
    import concourse.bass as bass
    import concourse.tile as tile
    from concourse import bacc, mybir

    KPART = os.environ.get("KPART", "all")

    f32 = mybir.dt.float32
    bf16 = mybir.dt.bfloat16
    i8 = mybir.dt.int8
    AF = mybir.ActivationFunctionType
    ALU = mybir.AluOpType

    f32r = mybir.dt.float32r

    SBW = NSUB * NB * B
    nc = bacc.Bacc(None, target_bir_lowering=False)
    gi8_d = nc.dram_tensor("gi8", [P, T_pad + SBW + W_S], i8,
                           kind="ExternalInput")
    radg_d = nc.dram_tensor("radg_T", [NR, T_pad], i8, kind="ExternalInput")
    MW = 2 * P + 9 + NSUB * NB + 1
    cmisc_d = nc.dram_tensor("cmisc", [P, MW], f32, kind="ExternalInput")
    CW = H + B * H + H + 8 * H
    cwx_d = nc.dram_tensor("cwx", [P, CW], bf16, kind="ExternalInput")
    out_d = nc.dram_tensor("out_T", [P, W_S], bf16, kind="ExternalOutput")

    with tile.TileContext(nc) as tc:
        with (
            tc.tile_pool(name="consts", bufs=1) as cp,
            tc.tile_pool(name="persist", bufs=1) as pp,
        ):
            cmisc_t = cp.tile([P, MW], f32)
            nc.gpsimd.dma_start(out=cmisc_t[:], in_=cmisc_d[:, :])
            cwx_t = cp.tile([P, CW], bf16)
            nc.gpsimd.dma_start(out=cwx_t[:], in_=cwx_d[:, :])
            sbf8_t = cp.tile([P, SBW], i8)
            nc.gpsimd.dma_start(out=sbf8_t[:], in_=gi8_d[:, T_pad:T_pad + SBW])
            sbf_f = cp.tile([P, SBW], f32)
            nc.vector.tensor_copy(out=sbf_f[:], in_=sbf8_t[:])
            x8_sb = cp.tile([P, W_S], i8)
            nc.gpsimd.dma_start(out=x8_sb[:],
                                in_=gi8_d[:, T_pad + SBW:T_pad + SBW + W_S])
            cwf_t = cp.tile([P, 8 * H], f32r)
            nc.vector.tensor_copy(out=cwf_t[:],
                                  in_=cwx_t[:, H + B * H + H:CW])
            aggT_big = pp.tile([P, W_S], f32)

            iota_t = cmisc_t[:, 0:P]
            ident_t = cmisc_t[:, P:2 * P]
            bias_t = cmisc_t[:, 2 * P:2 * P + 9]
            tol_t = cmisc_t[:, 2 * P + 9:2 * P + 9 + NSUB * NB]
            xsc_t = cmisc_t[:, MW - 1:MW]
            w_from_t = cwx_t[:, 0:H]
            W2_t = cwx_t[:, H:H + B * H]
            w_rbf_t = cwx_t[0:NR, H + B * H:H + B * H + H]
            epw_t = cwf_t
            b_from = bias_t[:, 0:1]

            # ---------------- main loop ----------------
            with (
                tc.tile_pool(name="mio", bufs=4) as mio,
                tc.tile_pool(name="mwork", bufs=3) as mwork,
                tc.tile_pool(name="ptmp", bufs=1, space="PSUM") as ptmp,
                tc.tile_pool(name="pxk", bufs=1, space="PSUM") as pxk,
                tc.tile_pool(name="pagg", bufs=2, space="PSUM") as pagg,
                tc.tile_pool(name="psmall", bufs=1, space="PSUM") as psmall,
            ):
                for b in range(NB if KPART in ("all", "main") else 0):
                    c0 = b * BLK_T
                    xg8 = mio.tile([P, BLK_T], i8, tag="xg8")
                    nc.gpsimd.dma_start(out=xg8[:], in_=gi8_d[:, c0:c0 + BLK_T])
                    rad8 = mio.tile([NR, BLK_T], i8, tag="rad8")
                    nc.gpsimd.dma_start(out=rad8[:], in_=radg_d[:, c0:c0 + BLK_T])
                    rad = mwork.tile([NR, BLK_T], bf16, tag="rad")
                    nc.vector.tensor_copy(out=rad[:], in_=rad8[:])
                    xgc = mwork.tile([P, BLK_T], bf16, tag="xgc")
                    nc.vector.tensor_copy(out=xgc[:], in_=xg8[:])

                    xkj_p = pxk.tile([P, BLK_T], f32, tag="xkj_p")
                    nc.tensor.matmul(out=xkj_p[:], lhsT=w_from_t, rhs=xgc[:],
                                     start=True, stop=True)
                    rbf_p = pxk.tile([P, BLK_T], f32, tag="rbf_p")
                    nc.tensor.matmul(out=rbf_p[:], lhsT=w_rbf_t, rhs=rad[:],
                                     start=True, stop=True)
                    xkj_s = mwork.tile([P, BLK_T], f32, tag="xkj_s")
                    nc.scalar.activation(out=xkj_s[:], in_=xkj_p[:], func=AF.Silu,
                                         bias=b_from, scale=1.0)
                    xkj = mwork.tile([P, BLK_T], bf16, tag="xkj")
                    nc.vector.tensor_tensor(out=xkj[:], in0=xkj_s[:], in1=rbf_p[:],
                                            op=ALU.mult)

                    agg_p = pagg.tile([P, P], f32, tag="agg")
                    for s in range(NSUB):
                        w0 = s * P
                        sc0 = (NSUB * b + s) * B
                        tmpA = ptmp.tile([P, 4 * H], f32, tag="tmpA")
                        nc.tensor.matmul(out=tmpA[:], lhsT=xkj[:, w0:w0 + P],
                                         rhs=W2_t[:, 0:4 * H], start=True, stop=True)
                        tmpB = ptmp.tile([P, 4 * H], f32, tag="tmpB")
                        nc.tensor.matmul(out=tmpB[:], lhsT=xkj[:, w0:w0 + P],
                                         rhs=W2_t[:, 4 * H:8 * H], start=True,
                                         stop=True)

                        S = mwork.tile([P, P], bf16, tag="S")
                        nc.vector.tensor_tensor(
                            out=S[:],
                            in0=tol_t[:, NSUB * b + s: NSUB * b + s + 1]
                                .to_broadcast([P, P]),
                            in1=iota_t, op=ALU.is_equal)
                        tmpS = mwork.tile([P, B * H], bf16, tag="tmpS")
                        for half, tsrc in ((0, tmpA), (1, tmpB)):
                            dst3 = tmpS[:, half * 4 * H:(half + 1) * 4 * H] \
                                .rearrange("p (b h) -> p b h", b=4)
                            src3 = tsrc[:].rearrange("p (b h) -> p b h", b=4)
                            sc3 = sbf_f[:, sc0 + half * 4:sc0 + half * 4 + 4] \
                                .unsqueeze(2).to_broadcast([P, 4, H])
                            nc.vector.tensor_tensor(out=dst3, in0=src3, in1=sc3,
                                                    op=ALU.mult)
                        for j in range(B):
                            nc.tensor.matmul(out=agg_p[:], lhsT=S[:],
                                             rhs=tmpS[:, j * H:(j + 1) * H],
                                             start=(s == 0 and j == 0),
                                             stop=(s == NSUB - 1 and j == B - 1),
                                             skip_group_check=True)
                    agg_s = mwork.tile([P, P], f32, tag="agg_s")
                    nc.scalar.activation(out=agg_s[:], in_=agg_p[:], func=AF.Copy)
                    aggT_p = psmall.tile([P, P], f32, tag="aggT_p")
                    nc.tensor.transpose(out=aggT_p[:], in_=agg_s[:],
                                        identity=ident_t)
                    nc.vector.tensor_copy(
                        out=aggT_big[:, b * SLOT_W:(b + 1) * SLOT_W],
                        in_=aggT_p[:])

            # -------- epilogue (baseline f32r structure, bf16 I/O) --------
            with (
                tc.tile_pool(name="ework", bufs=2) as ework,
                tc.tile_pool(name="epsum", bufs=4, space="PSUM") as epsum,
            ):
                def ep_mm(lhs_idx, rhs_ap, n):
                    pt = epsum.tile([P, EP_N], f32, tag="ep_p")
                    nc.tensor.matmul(out=pt[:, :n],
                                     lhsT=epw_t[:, lhs_idx * H:(lhs_idx + 1) * H],
                                     rhs=rhs_ap, start=True, stop=True)
                    return pt

                def ep_silu(pt, bias_idx, n, tag, dt_=f32):
                    t = ework.tile([P, EP_N], dt_, tag=tag)
                    nc.scalar.activation(out=t[:, :n], in_=pt[:, :n], func=AF.Silu,
                                         bias=bias_t[:, bias_idx:bias_idx + 1],
                                         scale=1.0)
                    return t

                n_ep = (W_S + EP_N - 1) // EP_N
                if KPART == "main":
                    for eb in range(n_ep):
                        c0 = eb * EP_N
                        n = min(EP_N, W_S - c0)
                        nc.gpsimd.dma_start(out=out_d[:, c0:c0 + n],
                                            in_=aggT_big[:, c0:c0 + n])
                    n_ep = 0
                for eb in range(n_ep):
                    c0 = eb * EP_N
                    n = min(EP_N, W_S - c0)
                    xf = ework.tile([P, EP_N], f32, tag="xf")
                    nc.vector.tensor_copy(out=xf[:, :n], in_=x8_sb[:, c0:c0 + n])
                    x_t = ework.tile([P, EP_N], f32r, tag="x_t")
                    nc.vector.tensor_tensor(out=x_t[:, :n], in0=xf[:, :n],
                                            in1=xsc_t.to_broadcast([P, n]),
                                            op=ALU.mult)
                    # h = silu(x@w_to+b_to) + agg
                    pt = ep_mm(0, x_t[:, :n], n)
                    xji = ep_silu(pt, 1, n, "xji")
                    h = ework.tile([P, EP_N], f32r, tag="h")
                    nc.vector.tensor_tensor(out=h[:, :n], in0=xji[:, :n],
                                            in1=aggT_big[:, c0:c0 + n], op=ALU.add)
                    # rb residual
                    t1 = ep_silu(ep_mm(1, h[:, :n], n), 2, n, "t1", f32r)
                    t2 = ep_silu(ep_mm(2, t1[:, :n], n), 3, n, "t2")
                    h2 = ework.tile([P, EP_N], f32r, tag="h2")
                    nc.vector.tensor_tensor(out=h2[:, :n], in0=h[:, :n],
                                            in1=t2[:, :n], op=ALU.add)
                    # lin + skip x
                    l1 = ep_silu(ep_mm(3, h2[:, :n], n), 4, n, "l1")
                    h3 = ework.tile([P, EP_N], f32r, tag="h3")
                    nc.vector.tensor_tensor(out=h3[:, :n], in0=l1[:, :n],
                                            in1=x_t[:, :n], op=ALU.add)
                    # ra residuals x2
                    t3 = ep_silu(ep_mm(4, h3[:, :n], n), 5, n, "t3", f32r)
                    t4 = ep_silu(ep_mm(5, t3[:, :n], n), 6, n, "t4")
                    h4 = ework.tile([P, EP_N], f32r, tag="h4")
                    nc.vector.tensor_tensor(out=h4[:, :n], in0=h3[:, :n],
                                            in1=t4[:, :n], op=ALU.add)
                    t5 = ep_silu(ep_mm(6, h4[:, :n], n), 7, n, "t5", f32r)
                    t6 = ep_silu(ep_mm(7, t5[:, :n], n), 8, n, "t6")
                    h5 = ework.tile([P, EP_N], bf16, tag="h5")
                    nc.vector.tensor_tensor(out=h5[:, :n], in0=h4[:, :n],
                                            in1=t6[:, :n], op=ALU.add)
                    nc.gpsimd.dma_start(out=out_d[:, c0:c0 + n], in_=h5[:, :n])
    nc.compile()
    return nc


def kernel(x, radial_basis, spherical_basis, edge_index_from, edge_index_to,
           w_rbf, w_sbf, w_from, b_from, w_to, b_to, W,
           rb_w, rb_b, lin_w, lin_b, ra_w, ra_b):
    from concourse.bass_utils import run_bass_kernel_spmd

    x = np.asarray(x, np.float32)
    radial = np.asarray(radial_basis, np.float32)
    sph = np.asarray(spherical_basis, np.float32)
    e_from = np.asarray(edge_index_from)
    e_to = np.asarray(edge_index_to)
    in_dtype = np.asarray(x).dtype

    x_scale = float(np.abs(x).max()) or 1.0
    rad_scale = float(np.abs(radial).max()) or 1.0
    sbf_scale_out = []
    cores, meta = host_prep(x, radial, sph, e_from, e_to,
                            np.asarray(w_sbf, np.float32), x_scale, rad_scale,
                            sbf_scale_out)
    sbf_scale = sbf_scale_out[0]
    NB, T_pad, W_S = meta['NB'], meta['T_pad'], meta['W_S']

    W_np = np.asarray(W, np.float32)
    W2 = np.ascontiguousarray(W_np.transpose(2, 1, 0).reshape(H, B * H))
    ep_w = np.concatenate([
        np.asarray(w_to, np.float32),
        np.asarray(rb_w, np.float32)[0, 0], np.asarray(rb_w, np.float32)[0, 1],
        np.asarray(lin_w, np.float32),
        np.asarray(ra_w, np.float32)[0, 0], np.asarray(ra_w, np.float32)[0, 1],
        np.asarray(ra_w, np.float32)[1, 0], np.asarray(ra_w, np.float32)[1, 1],
    ], axis=1)
    biases = np.stack([
        np.asarray(b_from, np.float32), np.asarray(b_to, np.float32),
        np.asarray(rb_b, np.float32)[0, 0], np.asarray(rb_b, np.float32)[0, 1],
        np.asarray(lin_b, np.float32),
        np.asarray(ra_b, np.float32)[0, 0], np.asarray(ra_b, np.float32)[0, 1],
        np.asarray(ra_b, np.float32)[1, 0], np.asarray(ra_b, np.float32)[1, 1],
    ], axis=1).astype(np.float32)
    iota = np.tile(np.arange(P, dtype=np.float32), (P, 1))
    ident = np.eye(P, dtype=np.float32)

    CW = H + B * H + H + 8 * H
    cw = np.zeros((P, CW), np.float32)
    cw[:, 0:H] = np.asarray(w_from, np.float32) * (x_scale / 127.0)
    cw[:, H:H + B * H] = W2 * (sbf_scale / 127.0)
    cw[0:NR, H + B * H:H + B * H + H] = \
        np.asarray(w_rbf, np.float32) * (rad_scale / 127.0)
    cw[:, H + B * H + H:CW] = ep_w
    cw16 = cw.astype(BF16)

    key = (NB, T_pad, W_S)
    nc = _PROG_CACHE.get(key)
    if nc is None:
        nc = build_program(NB, T_pad, W_S)
        _PROG_CACHE[key] = nc

    MW = 2 * P + 9 + NSUB * NB + 1
    in_maps = []
    for core in cores:
        cmisc = np.zeros((P, MW), np.float32)
        cmisc[:, 0:P] = iota
        cmisc[:, P:2 * P] = ident
        cmisc[:, 2 * P:2 * P + 9] = biases
        cmisc[:, 2 * P + 9:2 * P + 9 + NSUB * NB] = core['tol_cols']
        cmisc[:, MW - 1] = x_scale / 127.0
        in_maps.append({
            "gi8": core['gi8'], "radg_T": core['radg_T'],
            "cmisc": np.ascontiguousarray(cmisc),
            "cwx": np.ascontiguousarray(cw16),
        })
    res = run_bass_kernel_spmd(nc, in_maps, core_ids=list(range(N_CORES)))
    kernel._last_results = res
    if os.environ.get("KERNEL_EXEC_TWICE"):
        import time as _time
        os.environ["BASS_NEVER_TRACE"] = "1"
        try:
            t0 = _time.perf_counter()
            run_bass_kernel_spmd(nc, in_maps, core_ids=list(range(N_CORES)))
            kernel._exec2_s = _time.perf_counter() - t0
        finally:
            os.environ.pop("BASS_NEVER_TRACE", None)

    E_ = x.shape[0]
    out = np.zeros((E_, H), np.float32)
    for core, om in zip(cores, res.results):
        hT = np.asarray(om["out_T"], dtype=np.float32)
        for b in range(NB):
            lo, w = int(core['cov_lo'][b]), int(core['cov_w'][b])
            if w > 0:
                out[core['e_lo'] + lo: core['e_lo'] + lo + w] = \
                    hT[:, b * SLOT_W: b * SLOT_W + w].T
    return out.astype(in_dtype, copy=False)
